# revision 13
# baseline (speedup 1.0000x reference)
"""Trainium2 Bass kernel for nn_Conv1d_fft (B=16, Cin=Cout=128, L=4096, K=129, PAD=32).

The reference computes the conv via FFT with circular length 4160, output
truncated to 4032. Because 4160 >= L + 2*PAD and only the first 4032 samples
are kept, the circular wrap only ever touches zero padding, so the whole op
is exactly a plain cross-correlation (PyTorch-style Conv1d with padding=32)
plus a bias:

    out[b, o, n] = bias[o] + sum_{i, t} weight[o, i, t] * xp[b, i, n + t]

with xp = x zero-padded by 32 on each side (length 4160), n in [0, 4032).

Strategy: data-parallel over batch (2 batches per core, 8 cores). Per core,
the conv is 129 shifted matmuls accumulated in PSUM per output tile:
lhsT = weight[:, :, t] transposed to (Cin, Cout), rhs = xp window (Cin, 504).
Weight is pre-transposed on the host to (Cin, K*Cout) so DMA is contiguous.

Matmul dtype: float16 (default). fp16 carries 10 mantissa bits — the same
precision as TF32/f32r — while streaming at the full 16-bit rate with fast
(FWL) weight loads. Our data fits fp16's 5-bit exponent easily (x ~ N(0,1),
w in [-0.008, 0.008]; PSUM accumulation is always fp32), so f16 gives
f32r accuracy at bf16 speed.

Measured on trn2 (8 cores, NTFF profile): 458.6 us, rel err 2.85e-4 vs
the fp32 FFT reference. Breakdown: ~10.6 us DMA startup (prioritized
first-tile slices), 2064 matmuls at 214.3 ns each — exactly the PE stream
roofline for N=504 (504 cycles @ 2.4 GHz + NX dispatch), zero PE gaps —
and ~11.6 us Tile kernel-tail barrier. Alternatives measured: plain fp32
1894 us (7.5e-7); f32r 493.5 us (2.85e-4; streams at ~1.08 cyc/row on
silicon — confirmed by eliminating 87% of LDWEIGHTS via walrus
--enable-ldw-opt=true + tap-outer weight sharing with no spacing change);
bf16 473 us (2.1e-3, no accuracy benefit over f16 at the same speed).
"""

import os
import numpy as np

import concourse.bass as bass
import concourse.bacc as bacc
import concourse.tile as tile
import concourse.mybir as mybir
from concourse.bass_utils import run_bass_kernel_spmd

B, CIN, COUT, L, K = 16, 128, 128, 4096, 129
PAD = 32
OUT_LEN = 2 * PAD + L - (K - 1)  # 4032
LP = L + 2 * PAD                 # 4160
N_CORES = 8
BPC = B // N_CORES               # batches per core
TW = 504                         # output tile width (8 * 504 = 4032)
NT = OUT_LEN // TW
WCHUNKS = 8                      # weight DMA split (taps per chunk below)

F32 = mybir.dt.float32
BF16 = mybir.dt.bfloat16
F16 = mybir.dt.float16
F32R = mybir.dt.float32r

_cache = {}


def _tap_chunks():
    """Contiguous tap chunks for the weight DMA split. The first chunk is
    tiny so the very first matmuls wait on a ~0.26 MB transfer instead of
    ~1 MB; later chunks are bigger for DMA efficiency."""
    sizes = [4, 14, 18, 18, 19, 19, 19, 18]
    assert sum(sizes) == K and len(sizes) == WCHUNKS
    bounds = [0]
    for s in sizes:
        bounds.append(bounds[-1] + s)
    return [(bounds[i], bounds[i + 1]) for i in range(WCHUNKS)]


def _build_program(mode: str, order: str):
    """mode: f32 | f32r | bf16;  order: tap_inner | tap_outer."""
    io_dt = {"f32": F32, "f32r": F32R, "bf16": BF16, "f16": F16}[mode]
    nc = bacc.Bacc("TRN2", target_bir_lowering=False, debug=False,
                   num_devices=N_CORES)

    x_d = nc.dram_tensor("x", [BPC, CIN, LP], io_dt, kind="ExternalInput").ap()
    w_d = nc.dram_tensor("w", [CIN, K * COUT], io_dt, kind="ExternalInput").ap()
    b_d = nc.dram_tensor("b", [COUT, 1], F32, kind="ExternalInput").ap()
    o_d = nc.dram_tensor("out", [BPC, COUT, OUT_LEN], F32,
                         kind="ExternalOutput").ap()

    chunks = _tap_chunks()

    with tile.TileContext(nc) as tc:
        with (
            tc.tile_pool(name="wp", bufs=1) as wp,
            tc.tile_pool(name="xp", bufs=1) as xp,
            tc.tile_pool(name="bp", bufs=1) as bp,
            tc.tile_pool(name="op", bufs=4) as op,
            tc.tile_pool(name="ps", bufs=8 if order == "tap_inner" else 1,
                         space=bass.MemorySpace.PSUM) as ps,
        ):
            # DMA priority order: the first matmul group (batch 0, tile 0)
            # only needs x[0][:, :1136] and weight chunk 0, so issue those
            # first; the rest streams in behind while the PE is already busy.
            w_sb = [wp.tile([CIN, (t1 - t0) * COUT], io_dt, tag=f"w{ci}",
                            name=f"wsb{ci}")
                    for ci, (t0, t1) in enumerate(chunks)]
            x_sb = [xp.tile([CIN, LP], io_dt, tag=f"x{b}", name=f"xsb{b}")
                    for b in range(BPC)]

            # Critical path: tile (b=0, j=0) reads x[0][:, :632] and all taps.
            # Issue a small x slice, then weight chunks (smallest first), with
            # the rest of x interleaved behind the first two weight chunks.
            XA = TW + COUT      # columns needed by the first tile group
            XB = 2 * TW + COUT  # ... by the first two tile groups
            nc.sync.dma_start(x_sb[0][:, :XA], x_d[0][:, :XA])
            for ci, (t0, t1) in enumerate(chunks):
                nc.sync.dma_start(w_sb[ci][:], w_d[:, t0 * COUT:t1 * COUT])
                if ci == 0:
                    nc.sync.dma_start(x_sb[0][:, XA:XB], x_d[0][:, XA:XB])
                elif ci == 1:
                    nc.sync.dma_start(x_sb[0][:, XB:], x_d[0][:, XB:])
            b_sb = bp.tile([COUT, 1], F32)
            nc.sync.dma_start(b_sb[:], b_d[:])
            for b in range(1, BPC):
                nc.sync.dma_start(x_sb[b][:], x_d[b])

            def w_ap(t):
                for ci, (t0, t1) in enumerate(chunks):
                    if t0 <= t < t1:
                        return w_sb[ci][:, (t - t0) * COUT:(t - t0 + 1) * COUT]
                raise AssertionError

            def drain(psum_tile, b, j):
                o_sb = op.tile([COUT, TW], F32)
                nc.vector.tensor_scalar_add(o_sb[:], psum_tile[:], b_sb[:])
                nc.sync.dma_start(o_d[b][:, j * TW:(j + 1) * TW], o_sb[:])

            if order == "tap_inner":
                for b in range(BPC):
                    for j in range(NT):
                        acc = ps.tile([COUT, TW], F32)
                        for t in range(K):
                            nc.tensor.matmul(
                                acc[:],
                                w_ap(t),
                                x_sb[b][:, j * TW + t: j * TW + t + TW],
                                start=(t == 0), stop=(t == K - 1),
                            )
                        drain(acc, b, j)
            else:  # tap_outer
                for b in range(BPC):
                    accs = [ps.tile([COUT, TW], F32, tag=f"acc{j}", name=f"accs{j}")
                            for j in range(NT)]
                    for t in range(K):
                        for j in range(NT):
                            nc.tensor.matmul(
                                accs[j][:],
                                w_ap(t),
                                x_sb[b][:, j * TW + t: j * TW + t + TW],
                                start=(t == 0), stop=(t == K - 1),
                            )
                    for j in range(NT):
                        drain(accs[j], b, j)

    nc.compile()
    return nc


def _get_program(mode, order):
    key = (mode, order)
    if key not in _cache:
        _cache[key] = _build_program(mode, order)
    return _cache[key]


def _round_tf32(a: np.ndarray) -> np.ndarray:
    """Round fp32 to TF32 (10 mantissa bits), round-to-nearest-even."""
    u = np.ascontiguousarray(a, dtype=np.float32).view(np.uint32)
    r = (u + np.uint32(0xFFF) + ((u >> np.uint32(13)) & np.uint32(1))) \
        & np.uint32(0xFFFFE000)
    return r.view(np.float32)


def kernel(x, weight, bias, _trace=False, _trace_kwargs=None):
    mode = os.environ.get("BASS_CONV_MODE", "f16")
    order = os.environ.get("BASS_CONV_ORDER", "tap_inner")
    nc = _get_program(mode, order)

    if mode == "bf16":
        import ml_dtypes
        io_np = ml_dtypes.bfloat16
    elif mode == "f16":
        io_np = np.float16
    else:
        io_np = np.float32

    xp = np.zeros((B, CIN, LP), dtype=np.float32)
    xp[:, :, PAD:PAD + L] = x
    # (Cout, Cin, K) -> (Cin, K, Cout) so per-tap lhsT slices are contiguous
    wT = np.ascontiguousarray(np.transpose(
        np.asarray(weight, dtype=np.float32), (1, 2, 0)))
    if mode == "f32r":
        xp = _round_tf32(xp)
        wT = _round_tf32(wT)
    xp = np.ascontiguousarray(xp.astype(io_np))
    wT = np.ascontiguousarray(wT.astype(io_np)).reshape(CIN, K * COUT)
    b2 = np.ascontiguousarray(np.asarray(bias, np.float32).reshape(COUT, 1))

    in_maps = [
        {"x": xp[c * BPC:(c + 1) * BPC], "w": wT, "b": b2}
        for c in range(N_CORES)
    ]
    res = run_bass_kernel_spmd(
        nc, in_maps, list(range(N_CORES)),
        trace=_trace, **(_trace_kwargs or {}),
    )
    out = np.concatenate([res.results[c]["out"] for c in range(N_CORES)],
                         axis=0).astype(np.float32)
    if _trace:
        return out, res
    return out
